# revision 7
# baseline (speedup 1.0000x reference)
"""Multi-head linear attention (Performer/FAVOR+) Bass kernel for 8x TRN2 cores.

Sharding: 8 cores = 4 batches x 2 head-groups. Core c handles batch c//2 and
heads [4*(c%2), 4*(c%2)+4).

Math notes (exact rewrites of the reference, not approximations):
  - omega is sqrt(64) * orthogonal, so Omega @ Omega.T = 64*I. Hence
    0.5*||q||^2 = ||q @ Omega.T||^2 / 128: the squared-sum term is computed
    from xw itself and the plain q/k projections are never needed.
  - The per-row scale exp(-sq_t) on phi(q), the global 1/sqrt(128) scale and
    (approximately) the +EPS term all cancel in out = qkv[..,:64]/qkv[..,64],
    so the q-side feature map is just exp(+-xw).
  - The k-side scale rho_s = exp(-ksq_s) is folded into v1 = [v, 1]*rho so
    kp is also just exp(+-kxw).
  - The final divide (qkv[..,:64] / qkv[..,64]) runs on the HOST: the NEFF
    streams out raw [qkv_v | normalizer] per head in bf16.

Q-side projection computes only the 64 positive features per head (wqp2
packs two heads into the 128 stationary columns); exp(+x) and exp(-x) are
two full-partition ACT ops over the same PSUM tile, giving qpP=[h0+,h1+]
and qpM=[h0-,h1-] tiles. The qkv matmuls then use block-diagonal moving
operands kvBD_P/kvBD_M (built once per rep with cross-partition-offset ACT
copies) so one 128-contraction MM yields both heads' partial products.

Layouts: inputs are pre-transposed to f-major fp16 on the host. All
projections contract f=512 over 4 chunks of 128 partitions. PSUM fp32.

Schedule: phase-Q projection work (qxw matmuls + exp) is interleaved into the
KV s-loop to fill dependency stalls; the qkv raw-dump tail runs last.
PSUM budget (8 banks): kxwv(2x2) + qx2(2) + kvacc(1); tail qkv units reuse
the kxwv/qx2 tags (2 banks each, 3 rotating).
"""

import sys

import numpy as np

for _p in ("/opt/trn_rl_repo", "/root/.axon_site/_ro/trn_rl_repo"):
    try:
        import concourse  # noqa: F401
        break
    except ImportError:
        if _p not in sys.path:
            sys.path.insert(0, _p)

B, T, D, H = 4, 4096, 512, 8
DK = DV = 64
HPC = 4            # heads per core
NCH = 4            # f chunks (512 / 128)
P = 128
ST = T // P        # 32 s-tiles
TC = 8             # t chunks
TCW = T // TC      # 512

_CACHE = {}


def _build_program(reps=1):
    import concourse.mybir as mybir
    import concourse.tile as tile
    from concourse import bacc
    from contextlib import ExitStack

    dt = mybir.dt
    AF = mybir.ActivationFunctionType

    nc = bacc.Bacc("TRN2", target_bir_lowering=False, debug=False)

    qt_d = nc.dram_tensor("qt", [D, T], dt.float16, kind="ExternalInput")
    kt_d = nc.dram_tensor("kt", [D, T], dt.float16, kind="ExternalInput")
    vt_d = nc.dram_tensor("vt", [D, T], dt.float16, kind="ExternalInput")
    wqp2_d = nc.dram_tensor("wqp2", [2, NCH, P, P], dt.float16, kind="ExternalInput")
    wko_d = nc.dram_tensor("wko", [NCH, P, HPC * DK], dt.float16, kind="ExternalInput")
    wv_d = nc.dram_tensor("wv", [NCH, P, HPC * DV], dt.float16, kind="ExternalInput")
    # raw out: per t row, 2 head-pairs x 130 = [h_even v(64) | h_even n |
    # h_odd v(64) | h_odd n]
    out_d = nc.dram_tensor("out", [T * 2 * 130], dt.bfloat16, kind="ExternalOutput")

    with tile.TileContext(nc) as tc, ExitStack() as ctx:
        const = ctx.enter_context(tc.tile_pool(name="const", bufs=1))
        work = ctx.enter_context(tc.tile_pool(name="work", bufs=3))
        psum = ctx.enter_context(tc.tile_pool(name="psum", bufs=1, space="PSUM"))
        for _rep in range(reps):
            _emit_body(nc, tc, const, work, psum, mybir, dt, AF,
                       qt_d, kt_d, vt_d, wqp2_d, wko_d, wv_d, out_d)

    nc.compile()
    return nc


def _emit_body(nc, tc, const, work, psum, mybir, dt, AF,
               qt_d, kt_d, vt_d, wqp2_d, wko_d, wv_d, out_d):
    if True:

        # persistent SBUF residents
        qt = const.tile([P, NCH, T], dt.float16)
        kt = const.tile([P, NCH, T], dt.float16)
        vt = const.tile([P, NCH, T], dt.float16)
        wqp2 = const.tile([P, 2, NCH, P], dt.float16)
        wko = const.tile([P, NCH, HPC * DK], dt.float16)
        wv = const.tile([P, NCH, HPC * DV], dt.float16)
        kvbd_p = const.tile([P, 2, 130], dt.bfloat16)
        kvbd_m = const.tile([P, 2, 130], dt.bfloat16)

        # Coalesced loads: one DMA per (tensor, column window) spanning all
        # 4 f-chunks. Order matters: k/v first columns unblock pair 0,
        # wqp2/qt follow for the first q-chunk, then windows stream in the
        # order the loop consumes them.
        nc.sync.dma_start(out=wko[:], in_=wko_d.ap().rearrange("c p n -> p c n"))
        nc.sync.dma_start(out=wv[:], in_=wv_d.ap().rearrange("c p n -> p c n"))

        def load_win(dst, src_d, lo, hi, eng=None):
            (eng or nc.sync).dma_start(
                out=dst[:, :, lo:hi],
                in_=src_d.ap()[:, lo:hi].rearrange("(c p) w -> p c w", p=P),
            )

        def load_chunk(dst, src_d, c, lo, hi, eng=None):
            (eng or nc.sync).dma_start(
                out=dst[:, c, lo:hi],
                in_=src_d[c * P:(c + 1) * P, lo:hi],
            )

        # First s-window split per f-chunk and spread across two issue
        # queues: each chunk lands on its own DMA queue, and the c=0 matmul
        # can start as soon as chunk 0 arrives.
        for c in range(NCH):
            load_chunk(kt, kt_d, c, 0, 256)
            load_chunk(vt, vt_d, c, 0, 256, eng=nc.gpsimd)
        nc.sync.dma_start(out=wqp2[:],
                          in_=wqp2_d.ap().rearrange("g c p m -> p g c m"))
        load_win(qt, qt_d, 0, 512, eng=nc.gpsimd)
        load_win(kt, kt_d, 256, 512)
        load_win(vt, vt_d, 256, 512, eng=nc.gpsimd)
        for lo, hi in ((512, 1536), (1536, 2560), (2560, 3584), (3584, T)):
            load_win(kt, kt_d, lo, hi)
            load_win(vt, vt_d, lo, hi, eng=nc.gpsimd)
            load_win(qt, qt_d, lo, hi)

        nc.vector.memset(kvbd_p[:], 0.0)
        nc.vector.memset(kvbd_m[:], 0.0)

        # Single-bank PSUM accumulator: kv[h] at columns [h*65, h*65+65).
        # first_mm clears has_written at BANK granularity, so interleaved
        # per-head groups must NOT use start=True: memset the bank once and
        # accumulate from the first matmul.
        kv_big = psum.tile([P, HPC, DV + 1], dt.float32, tag="kvacc", bufs=1)
        nc.vector.memset(kv_big[:], 0.0)

        qp_tiles = []

        def emit_q_chunk(tcx):
            tsl = slice(tcx * TCW, (tcx + 1) * TCW)
            qx2 = psum.tile([P, 2, TCW], dt.float32, tag="qx2", bufs=1,
                            name="qx2")
            for hp in range(2):
                for c in range(NCH):
                    nc.tensor.matmul(
                        qx2[:, hp, :], wqp2[:, hp, c, :], qt[:, c, tsl],
                        start=(c == 0), stop=(c == NCH - 1),
                    )
            qpp = work.tile([P, 2, TCW], dt.bfloat16, tag="qpp", bufs=TC,
                            name=f"qpp{tcx}")
            qpm = work.tile([P, 2, TCW], dt.bfloat16, tag="qpm", bufs=TC,
                            name=f"qpm{tcx}")
            nc.scalar.activation(qpp[:], qx2[:], AF.Exp, scale=1.0)
            nc.scalar.activation(qpm[:], qx2[:], AF.Exp, scale=-1.0)
            qp_tiles.append((qpp, qpm))

        # ---------------- phase KV (with q-projection work interleaved) -----
        # Engines execute their queues IN ORDER, so cross-engine dependencies
        # are software-pipelined: v1 (needs rho from ACT) is emitted one pair
        # late on DVE, the kv matmuls (need v1) one pair later still on PE.
        NP_ = ST // 2    # 16 pairs
        stage = {}       # pi -> dict of tiles

        def emit_v1(pi):
            st_ = stage[pi]
            v1 = work.tile([P, 2, HPC, DV + 1], dt.bfloat16, tag="v1",
                           name="v1")
            nc.vector.tensor_mul(
                v1[:, :, :, 0:DV], st_["v_ps"],
                st_["rho"][:].broadcast_to([P, 2, HPC, DV])
            )
            nc.vector.tensor_copy(v1[:, :, :, DV:DV + 1], st_["rho"][:])
            st_["v1"] = v1

        def emit_kv(pi):
            st_ = stage.pop(pi)
            for p_ in range(2):
                si = 2 * pi + p_
                for h in range(HPC):
                    nc.tensor.matmul(
                        kv_big[:, h, :], st_["kp"][:, p_, h, :],
                        st_["v1"][:, p_, h, :],
                        start=False, stop=(si == ST - 1),
                        skip_group_check=True,
                    )

        for pi in range(NP_):
            # kxw and v share PSUM banks: [..., 0:64] = kxw, 64:128 = v
            kxwv = psum.tile([P, 2, HPC, 2 * DK], dt.float32, tag="kxwv",
                             bufs=2, name="kxwv")
            kxw = kxwv[:, :, :, 0:DK]
            v_ps = kxwv[:, :, :, DK:2 * DK]
            for p_ in range(2):
                ssl = slice((2 * pi + p_) * P, (2 * pi + p_ + 1) * P)
                for c in range(NCH):
                    nc.tensor.matmul(
                        kxwv[:, p_, :, 0:DK], kt[:, c, ssl], wko[:, c, :],
                        start=(c == 0), stop=(c == NCH - 1),
                    )
                for c in range(NCH):
                    nc.tensor.matmul(
                        kxwv[:, p_, :, DK:2 * DK], vt[:, c, ssl], wv[:, c, :],
                        start=(c == 0), stop=(c == NCH - 1),
                    )
            if pi >= 2:
                emit_kv(pi - 2)

            kp = work.tile([P, 2, HPC, 2 * DK], dt.bfloat16, tag="kp", bufs=3)
            nc.scalar.activation(kp[:, :, :, 0:DK], kxw, AF.Exp, scale=1.0)
            nc.scalar.activation(kp[:, :, :, DK:2 * DK], kxw, AF.Exp,
                                 scale=-1.0)

            kxw_sb = work.tile([P, 2, HPC, DK], dt.bfloat16, tag="kxwsb",
                               bufs=2)
            nc.vector.tensor_copy(kxw_sb[:], kxw)
            sqsc = work.tile([P, 2, HPC, DK], dt.bfloat16, tag="sqsc", bufs=2)
            nc.vector.tensor_mul(sqsc[:], kxw_sb[:], kxw_sb[:])
            ksqr = work.tile([P, 2, HPC, 1], dt.float32, tag="ksqr")
            nc.vector.reduce_sum(ksqr[:], sqsc[:], axis=mybir.AxisListType.X)
            rho = work.tile([P, 2, HPC, 1], dt.float32, tag="rho")
            nc.scalar.activation(rho[:], ksqr[:], AF.Exp, scale=-1.0 / 128.0)

            stage[pi] = {"v_ps": v_ps, "rho": rho, "kp": kp}
            if pi >= 1:
                emit_v1(pi - 1)

            if pi % 2 == 1:
                emit_q_chunk(pi // 2)

        emit_v1(NP_ - 1)
        emit_kv(NP_ - 2)
        emit_kv(NP_ - 1)

        # Block-diagonal kv for the 2-head qkv matmuls. kv_big rows: 0:64 =
        # plus feats, 64:128 = minus feats; columns per head. ACT copies
        # support cross-partition-offset placement.
        for hp in range(2):
            nc.scalar.copy(kvbd_p[0:64, hp, 0:65], kv_big[0:64, 2 * hp, :])
            nc.scalar.copy(kvbd_p[64:128, hp, 65:130],
                           kv_big[0:64, 2 * hp + 1, :])
            nc.scalar.copy(kvbd_m[0:64, hp, 0:65], kv_big[64:128, 2 * hp, :])
            nc.scalar.copy(kvbd_m[64:128, hp, 65:130],
                           kv_big[64:128, 2 * hp + 1, :])

        # ---------------- tail: qkv raw dump (normalize on host) ------------
        # Units of 2 t-tiles: psum [128, 2, 2, 130] = 2 banks, rotating
        # through the kxwv(x2) + qx2 tags.
        unit_tags = [("kxwv", 2), ("kxwv", 2), ("qx2", 1)]
        ui = 0
        for tcx in range(TC):
            qpp, qpm = qp_tiles[tcx]
            for u in range(2):
                tg, bufs_ = unit_tags[ui % 3]
                ui += 1
                # regions padded to 256 fp32 so none crosses a PSUM bank
                # boundary (start=True clears has_written only for the bank
                # containing the write start; a region spilling into the
                # next bank would accumulate onto stale data on buf reuse)
                qkv = psum.tile([P, 2, 2, 256], dt.float32, tag=tg,
                                bufs=bufs_, name="qkv")
                for tt2 in range(2):
                    ttsl = slice((u * 2 + tt2) * P, (u * 2 + tt2 + 1) * P)
                    for hp in range(2):
                        nc.tensor.matmul(
                            qkv[:, tt2, hp, 0:130], qpp[:, hp, ttsl],
                            kvbd_p[:, hp, :], start=True, stop=False,
                        )
                        nc.tensor.matmul(
                            qkv[:, tt2, hp, 0:130], qpm[:, hp, ttsl],
                            kvbd_m[:, hp, :], start=False, stop=True,
                        )
                o_sb = work.tile([P, 2, 2, 130], dt.bfloat16, tag="osb",
                                 bufs=6)
                nc.vector.tensor_copy(o_sb[:], qkv[:, :, :, 0:130])
                base = (tcx * 512 + u * 256) * 260
                out_eng = nc.gpsimd if (ui % 2 == 0) else nc.sync
                out_eng.dma_start(
                    out=out_d.ap()[base:base + 256 * 260].rearrange(
                        "(tt2 p c) -> p tt2 c", tt2=2, p=P
                    ),
                    in_=o_sb[:],
                )


def _get_program(reps=1):
    if reps not in _CACHE:
        _CACHE[reps] = _build_program(reps)
    return _CACHE[reps]


def _prep_core_inputs(query, value, key, wqo, wko, wv_w, core):
    b, hg = core // 2, core % 2
    hs = slice(hg * HPC, (hg + 1) * HPC)

    qT = np.ascontiguousarray(query[b].T.astype(np.float16))   # (512, 4096)
    kT = np.ascontiguousarray(key[b].T.astype(np.float16))
    vT = np.ascontiguousarray(value[b].T.astype(np.float16))

    wqo_c = wqo[hs]                                            # (4, 512, 64)
    # head-pair packing: [h_even 64 cols | h_odd 64 cols]
    wqp2 = np.stack([
        np.concatenate([wqo_c[2 * hp], wqo_c[2 * hp + 1]], axis=1)
        for hp in range(2)
    ])                                                         # (2, 512, 128)
    wqp2 = np.ascontiguousarray(
        wqp2.reshape(2, NCH, P, P).astype(np.float16))         # (hp, c, p, m)

    wko_c = np.concatenate(list(wko[hs]), axis=1)              # (512, 256)
    wko_c = np.ascontiguousarray(
        wko_c.reshape(NCH, P, HPC * DK).astype(np.float16))
    wv_c = np.concatenate(list(wv_w[hs]), axis=1)              # (512, 256)
    wv_c = np.ascontiguousarray(
        wv_c.reshape(NCH, P, HPC * DV).astype(np.float16))

    return {"qt": qT, "kt": kT, "vt": vT,
            "wqp2": wqp2, "wko": wko_c, "wv": wv_c}


def kernel(query, value, key, wq, wv, wk, omega):
    from concourse.bass_utils import run_bass_kernel_spmd

    query = np.asarray(query, np.float32)
    value = np.asarray(value, np.float32)
    key = np.asarray(key, np.float32)
    wq = np.asarray(wq, np.float32)
    wv = np.asarray(wv, np.float32)
    wk = np.asarray(wk, np.float32)
    omega = np.asarray(omega, np.float32)

    nc = _get_program()

    wqo = np.einsum("hfk,mk->hfm", wq, omega)                  # (8, 512, 64)
    wko = np.einsum("hfk,mk->hfm", wk, omega)

    in_maps = [
        _prep_core_inputs(query, value, key, wqo, wko, wv, core)
        for core in range(8)
    ]
    res = run_bass_kernel_spmd(nc, in_maps, core_ids=list(range(8)))

    out = np.empty((B, T, D), np.float32)
    for core in range(8):
        b, hg = core // 2, core % 2
        raw = np.asarray(res.results[core]["out"], np.float32)
        raw = raw.reshape(T, 2, 130)                           # (t, hp, 130)
        # head h_local = 2*hp + j lives at cols [j*65, j*65+65)
        ov = np.empty((HPC, T, DV), np.float32)
        for hl in range(HPC):
            hp, j = hl // 2, hl % 2
            blk = raw[:, hp, j * 65:(j + 1) * 65]
            ov[hl] = blk[:, 0:DV] / blk[:, DV:DV + 1]
        out[b, hg * 2048:(hg + 1) * 2048, :] = ov.reshape(2048, 512)
    return out


# revision 8
# speedup vs baseline: 1.0632x; 1.0632x over previous
"""Multi-head linear attention (Performer/FAVOR+) Bass kernel for 8x TRN2 cores.

Sharding: 8 cores = 4 batches x 2 head-groups. Core c handles batch c//2 and
heads [4*(c%2), 4*(c%2)+4).

Math notes (exact rewrites of the reference, not approximations):
  - omega is sqrt(64) * orthogonal, so Omega @ Omega.T = 64*I. Hence
    0.5*||q||^2 = ||q @ Omega.T||^2 / 128: the squared-sum term is computed
    from xw itself and the plain q/k projections are never needed.
  - The per-row scale exp(-sq_t) on phi(q), the global 1/sqrt(128) scale and
    (approximately) the +EPS term all cancel in out = qkv[..,:64]/qkv[..,64],
    so the q-side feature map is just exp(+-xw).
  - The k-side scale rho_s = exp(-ksq_s) is folded into v1 = [v, 1]*rho so
    kp is also just exp(+-kxw).
  - The final divide (qkv[..,:64] / qkv[..,64]) runs on the HOST: the NEFF
    streams out raw [qkv_v | normalizer] per head in bf16.

Q-side projection computes only the 64 positive features per head (wqp2
packs two heads into the 128 stationary columns); exp(+x) and exp(-x) are
two full-partition ACT ops over the same PSUM tile, giving qpP=[h0+,h1+]
and qpM=[h0-,h1-] tiles. The qkv matmuls then use block-diagonal moving
operands kvBD_P/kvBD_M (built once per rep with cross-partition-offset ACT
copies) so one 128-contraction MM yields both heads' partial products.

DMA: descriptor generation on the issuing engine queue costs ~5ns per
descriptor, and a descriptor needs contiguous runs on BOTH ends. All DRAM
layouts are therefore window-blocked [p, c, w] (one contiguous run per
partition per DMA = 128 descriptors); SBUF tiles are flat per-partition
with computed column offsets. Loads split across the sync and gpsimd
queues. Output goes out in 4 batched p-major DMAs.

PSUM budget (8 banks): kxwv(2x2) + qx2(2) + kvacc(1); tail qkv units reuse
the kxwv/qx2 tags (2 banks each, 3 rotating, regions padded to bank size).
"""

import sys

import numpy as np

for _p in ("/opt/trn_rl_repo", "/root/.axon_site/_ro/trn_rl_repo"):
    try:
        import concourse  # noqa: F401
        break
    except ImportError:
        if _p not in sys.path:
            sys.path.insert(0, _p)

B, T, D, H = 4, 4096, 512, 8
DK = DV = 64
HPC = 4            # heads per core
NCH = 4            # f chunks (512 / 128)
P = 128
ST = T // P        # 32 s-tiles
TC = 8             # t chunks
TCW = T // TC      # 512

# window-blocked DRAM/SBUF layouts (shared host/kernel contract)
KWINS = [(0, 256), (256, 512), (512, 1536), (1536, 2560), (2560, 3584),
         (3584, 4096)]
QWINS = [(0, 512), (512, 1536), (1536, 2560), (2560, 3584), (3584, 4096)]


def _win_off(wins):
    offs = []
    base = 0
    for lo, hi in wins:
        offs.append(base)
        base += NCH * (hi - lo)
    return offs, base


KOFFS, KTOT = _win_off(KWINS)   # per-partition elems: 4*4096
QOFFS, QTOT = _win_off(QWINS)


def _col(wins, offs, c, col):
    """SBUF flat column offset for (chunk c, original column col)."""
    for (lo, hi), base in zip(wins, offs):
        if lo <= col < hi:
            return base + c * (hi - lo) + (col - lo)
    raise ValueError(col)


_CACHE = {}


def _build_program(reps=1):
    import concourse.mybir as mybir
    import concourse.tile as tile
    from concourse import bacc
    from contextlib import ExitStack

    dt = mybir.dt
    AF = mybir.ActivationFunctionType

    nc = bacc.Bacc("TRN2", target_bir_lowering=False, debug=False)

    qt_d = nc.dram_tensor("qt", [P * QTOT], dt.float16, kind="ExternalInput")
    kt_d = nc.dram_tensor("kt", [P * KTOT], dt.float16, kind="ExternalInput")
    vt_d = nc.dram_tensor("vt", [P * KTOT], dt.float16, kind="ExternalInput")
    wqp2_d = nc.dram_tensor("wqp2", [P * 2 * NCH * P], dt.float16,
                            kind="ExternalInput")
    wko_d = nc.dram_tensor("wko", [P * NCH * HPC * DK], dt.float16,
                           kind="ExternalInput")
    wv_d = nc.dram_tensor("wv", [P * NCH * HPC * DV], dt.float16,
                          kind="ExternalInput")
    # raw out, p-major: [p, unit(16), tt2(2), hp(2), 130] bf16
    out_d = nc.dram_tensor("out", [P * 16 * 520], dt.bfloat16,
                           kind="ExternalOutput")

    with tile.TileContext(nc) as tc, ExitStack() as ctx:
        const = ctx.enter_context(tc.tile_pool(name="const", bufs=1))
        work = ctx.enter_context(tc.tile_pool(name="work", bufs=3))
        psum = ctx.enter_context(tc.tile_pool(name="psum", bufs=1, space="PSUM"))
        for _rep in range(reps):
            _emit_body(nc, tc, const, work, psum, mybir, dt, AF,
                       qt_d, kt_d, vt_d, wqp2_d, wko_d, wv_d, out_d)

    nc.compile()
    return nc


def _emit_body(nc, tc, const, work, psum, mybir, dt, AF,
               qt_d, kt_d, vt_d, wqp2_d, wko_d, wv_d, out_d):
    if True:

        # persistent SBUF residents (flat window-blocked layouts)
        qt = const.tile([P, QTOT], dt.float16)
        kt = const.tile([P, KTOT], dt.float16)
        vt = const.tile([P, KTOT], dt.float16)
        wqp2 = const.tile([P, 2 * NCH * P], dt.float16)
        wko = const.tile([P, NCH * HPC * DK], dt.float16)
        wv = const.tile([P, NCH * HPC * DV], dt.float16)
        kvbd_p = const.tile([P, 2, 130], dt.bfloat16)
        kvbd_m = const.tile([P, 2, 130], dt.bfloat16)

        def flat_load(eng, dst, src_d, elem_off, elems):
            eng.dma_start(
                out=dst[:, elem_off:elem_off + elems],
                in_=src_d.ap()[P * elem_off:P * (elem_off + elems)].rearrange(
                    "(p x) -> p x", p=P),
            )

        # load order: weights and the first k/v windows unblock pair 0;
        # wqp2/qt follow for the first q-chunk; later windows stream in
        # consumption order, split across the sync and gpsimd queues.
        flat_load(nc.sync, wko, wko_d, 0, NCH * HPC * DK)
        flat_load(nc.gpsimd, wv, wv_d, 0, NCH * HPC * DV)
        kw = [NCH * (hi - lo) for lo, hi in KWINS]
        qw = [NCH * (hi - lo) for lo, hi in QWINS]
        flat_load(nc.sync, kt, kt_d, KOFFS[0], kw[0])
        flat_load(nc.gpsimd, vt, vt_d, KOFFS[0], kw[0])
        flat_load(nc.sync, wqp2, wqp2_d, 0, 2 * NCH * P)
        flat_load(nc.gpsimd, qt, qt_d, QOFFS[0], qw[0])
        flat_load(nc.sync, kt, kt_d, KOFFS[1], kw[1])
        flat_load(nc.gpsimd, vt, vt_d, KOFFS[1], kw[1])
        for wi in range(2, len(KWINS)):
            flat_load(nc.sync, kt, kt_d, KOFFS[wi], kw[wi])
            flat_load(nc.gpsimd, vt, vt_d, KOFFS[wi], kw[wi])
            flat_load(nc.sync if wi % 2 else nc.gpsimd,
                      qt, qt_d, QOFFS[wi - 1], qw[wi - 1])

        nc.vector.memset(kvbd_p[:], 0.0)
        nc.vector.memset(kvbd_m[:], 0.0)

        # Single-bank PSUM accumulator: kv[h] at columns [h*65, h*65+65).
        # first_mm clears has_written at BANK granularity, so interleaved
        # per-head groups must NOT use start=True: memset the bank once and
        # accumulate from the first matmul.
        kv_big = psum.tile([P, HPC, DV + 1], dt.float32, tag="kvacc", bufs=1)
        nc.vector.memset(kv_big[:], 0.0)

        qp_tiles = []

        def emit_q_chunk(tcx):
            qx2 = psum.tile([P, 2, TCW], dt.float32, tag="qx2", bufs=1,
                            name="qx2")
            for hp in range(2):
                for c in range(NCH):
                    qoff = _col(QWINS, QOFFS, c, tcx * TCW)
                    nc.tensor.matmul(
                        qx2[:, hp, :],
                        wqp2[:, (hp * NCH + c) * P:(hp * NCH + c + 1) * P],
                        qt[:, qoff:qoff + TCW],
                        start=(c == 0), stop=(c == NCH - 1),
                    )
            qpp = work.tile([P, 2, TCW], dt.bfloat16, tag="qpp", bufs=TC,
                            name=f"qpp{tcx}")
            qpm = work.tile([P, 2, TCW], dt.bfloat16, tag="qpm", bufs=TC,
                            name=f"qpm{tcx}")
            nc.scalar.activation(qpp[:], qx2[:], AF.Exp, scale=1.0)
            nc.scalar.activation(qpm[:], qx2[:], AF.Exp, scale=-1.0)
            qp_tiles.append((qpp, qpm))

        # ---------------- phase KV (with q-projection work interleaved) -----
        # Engines execute their queues IN ORDER, so cross-engine dependencies
        # are software-pipelined: v1 (needs rho from ACT) is emitted one pair
        # late on DVE, the kv matmuls (need v1) one pair later still on PE.
        NP_ = ST // 2    # 16 pairs
        stage = {}       # pi -> dict of tiles

        def emit_v1(pi):
            st_ = stage[pi]
            v1 = work.tile([P, 2, HPC, DV + 1], dt.bfloat16, tag="v1",
                           name="v1")
            nc.vector.tensor_mul(
                v1[:, :, :, 0:DV], st_["v_ps"],
                st_["rho"][:].broadcast_to([P, 2, HPC, DV])
            )
            nc.vector.tensor_copy(v1[:, :, :, DV:DV + 1], st_["rho"][:])
            st_["v1"] = v1

        def emit_kv(pi):
            st_ = stage.pop(pi)
            for p_ in range(2):
                si = 2 * pi + p_
                for h in range(HPC):
                    nc.tensor.matmul(
                        kv_big[:, h, :], st_["kp"][:, p_, h, :],
                        st_["v1"][:, p_, h, :],
                        start=False, stop=(si == ST - 1),
                        skip_group_check=True,
                    )

        for pi in range(NP_):
            # kxw and v share PSUM banks: [..., 0:64] = kxw, 64:128 = v
            kxwv = psum.tile([P, 2, HPC, 2 * DK], dt.float32, tag="kxwv",
                             bufs=2, name="kxwv")
            kxw = kxwv[:, :, :, 0:DK]
            v_ps = kxwv[:, :, :, DK:2 * DK]
            for p_ in range(2):
                scol = (2 * pi + p_) * P
                for c in range(NCH):
                    koff = _col(KWINS, KOFFS, c, scol)
                    nc.tensor.matmul(
                        kxwv[:, p_, :, 0:DK], kt[:, koff:koff + P],
                        wko[:, c * 256:(c + 1) * 256],
                        start=(c == 0), stop=(c == NCH - 1),
                    )
                for c in range(NCH):
                    koff = _col(KWINS, KOFFS, c, scol)
                    nc.tensor.matmul(
                        kxwv[:, p_, :, DK:2 * DK], vt[:, koff:koff + P],
                        wv[:, c * 256:(c + 1) * 256],
                        start=(c == 0), stop=(c == NCH - 1),
                    )
            if pi >= 2:
                emit_kv(pi - 2)

            kp = work.tile([P, 2, HPC, 2 * DK], dt.bfloat16, tag="kp", bufs=3)
            nc.scalar.activation(kp[:, :, :, 0:DK], kxw, AF.Exp, scale=1.0)
            nc.scalar.activation(kp[:, :, :, DK:2 * DK], kxw, AF.Exp,
                                 scale=-1.0)

            kxw_sb = work.tile([P, 2, HPC, DK], dt.bfloat16, tag="kxwsb",
                               bufs=2)
            nc.vector.tensor_copy(kxw_sb[:], kxw)
            sqsc = work.tile([P, 2, HPC, DK], dt.bfloat16, tag="sqsc", bufs=2)
            nc.vector.tensor_mul(sqsc[:], kxw_sb[:], kxw_sb[:])
            ksqr = work.tile([P, 2, HPC, 1], dt.float32, tag="ksqr")
            nc.vector.reduce_sum(ksqr[:], sqsc[:], axis=mybir.AxisListType.X)
            rho = work.tile([P, 2, HPC, 1], dt.float32, tag="rho")
            nc.scalar.activation(rho[:], ksqr[:], AF.Exp, scale=-1.0 / 128.0)

            stage[pi] = {"v_ps": v_ps, "rho": rho, "kp": kp}
            if pi >= 1:
                emit_v1(pi - 1)

            if pi % 2 == 1:
                emit_q_chunk(pi // 2)

        emit_v1(NP_ - 1)
        emit_kv(NP_ - 2)
        emit_kv(NP_ - 1)

        # Block-diagonal kv for the 2-head qkv matmuls. kv_big rows: 0:64 =
        # plus feats, 64:128 = minus feats; columns per head. ACT copies
        # support cross-partition-offset placement.
        for hp in range(2):
            nc.scalar.copy(kvbd_p[0:64, hp, 0:65], kv_big[0:64, 2 * hp, :])
            nc.scalar.copy(kvbd_p[64:128, hp, 65:130],
                           kv_big[0:64, 2 * hp + 1, :])
            nc.scalar.copy(kvbd_m[0:64, hp, 0:65], kv_big[64:128, 2 * hp, :])
            nc.scalar.copy(kvbd_m[64:128, hp, 65:130],
                           kv_big[64:128, 2 * hp + 1, :])

        # ---------------- tail: qkv raw dump (normalize on host) ------------
        # Units of 2 t-tiles: psum [128, 2, 2, 256] = 2 banks, rotating
        # through the kxwv(x2) + qx2 tags; regions padded to 256 fp32 so
        # none crosses a PSUM bank boundary (start=True clears has_written
        # only for the bank containing the write start). Output stages into
        # 4-unit SBUF groups, DMA'd p-major (128 descriptors each).
        unit_tags = [("kxwv", 2), ("kxwv", 2), ("qx2", 1)]
        for tcx in range(TC):
            qpp, qpm = qp_tiles[tcx]
            for u in range(2):
                ui = tcx * 2 + u
                tg, bufs_ = unit_tags[ui % 3]
                qkv = psum.tile([P, 2, 2, 256], dt.float32, tag=tg,
                                bufs=bufs_, name="qkv")
                for tt2 in range(2):
                    ttsl = slice((u * 2 + tt2) * P, (u * 2 + tt2 + 1) * P)
                    for hp in range(2):
                        nc.tensor.matmul(
                            qkv[:, tt2, hp, 0:130], qpp[:, hp, ttsl],
                            kvbd_p[:, hp, :], start=True, stop=False,
                        )
                        nc.tensor.matmul(
                            qkv[:, tt2, hp, 0:130], qpm[:, hp, ttsl],
                            kvbd_m[:, hp, :], start=False, stop=True,
                        )
                if ui % 4 == 0:
                    o_sb = work.tile([P, 4, 2, 2, 130], dt.bfloat16,
                                     tag="osb4", bufs=2, name="osb4")
                nc.vector.tensor_copy(o_sb[:, ui % 4, :, :, :],
                                      qkv[:, :, :, 0:130])
                if ui % 4 == 3:
                    g = ui // 4
                    out_eng = nc.gpsimd if g % 2 == 0 else nc.sync
                    out_eng.dma_start(
                        out=out_d.ap().rearrange(
                            "(p u x) -> p u x", p=P, u=16
                        )[:, g * 4:(g + 1) * 4, :],
                        in_=o_sb[:],
                    )


def _get_program(reps=1):
    if reps not in _CACHE:
        _CACHE[reps] = _build_program(reps)
    return _CACHE[reps]


def _win_block(a2d, wins):
    """[512, T] f-major -> concat of per-window [p, c, w] blocks, flat."""
    a4 = a2d.reshape(NCH, P, a2d.shape[1])
    blocks = [
        np.ascontiguousarray(a4[:, :, lo:hi].transpose(1, 0, 2)).ravel()
        for lo, hi in wins
    ]
    return np.concatenate(blocks)


def _prep_core_inputs(query, value, key, wqo, wko, wv_w, core):
    b, hg = core // 2, core % 2
    hs = slice(hg * HPC, (hg + 1) * HPC)

    qT = query[b].T.astype(np.float16)                         # (512, 4096)
    kT = key[b].T.astype(np.float16)
    vT = value[b].T.astype(np.float16)

    wqo_c = wqo[hs]                                            # (4, 512, 64)
    # head-pair packing: [h_even 64 cols | h_odd 64 cols]
    wqp2 = np.stack([
        np.concatenate([wqo_c[2 * hp], wqo_c[2 * hp + 1]], axis=1)
        for hp in range(2)
    ])                                                         # (2, 512, 128)
    # -> [p, hp, c, m] flat
    wqp2 = np.ascontiguousarray(
        wqp2.reshape(2, NCH, P, P).transpose(2, 0, 1, 3)).ravel()
    wqp2 = wqp2.astype(np.float16)

    wko_c = np.concatenate(list(wko[hs]), axis=1)              # (512, 256)
    wko_c = np.ascontiguousarray(
        wko_c.reshape(NCH, P, HPC * DK).transpose(1, 0, 2)).ravel()
    wko_c = wko_c.astype(np.float16)
    wv_c = np.concatenate(list(wv_w[hs]), axis=1)              # (512, 256)
    wv_c = np.ascontiguousarray(
        wv_c.reshape(NCH, P, HPC * DV).transpose(1, 0, 2)).ravel()
    wv_c = wv_c.astype(np.float16)

    return {"qt": _win_block(qT, QWINS), "kt": _win_block(kT, KWINS),
            "vt": _win_block(vT, KWINS),
            "wqp2": wqp2, "wko": wko_c, "wv": wv_c}


def kernel(query, value, key, wq, wv, wk, omega):
    from concourse.bass_utils import run_bass_kernel_spmd

    query = np.asarray(query, np.float32)
    value = np.asarray(value, np.float32)
    key = np.asarray(key, np.float32)
    wq = np.asarray(wq, np.float32)
    wv = np.asarray(wv, np.float32)
    wk = np.asarray(wk, np.float32)
    omega = np.asarray(omega, np.float32)

    nc = _get_program()

    wqo = np.einsum("hfk,mk->hfm", wq, omega)                  # (8, 512, 64)
    wko = np.einsum("hfk,mk->hfm", wk, omega)

    in_maps = [
        _prep_core_inputs(query, value, key, wqo, wko, wv, core)
        for core in range(8)
    ]
    res = run_bass_kernel_spmd(nc, in_maps, core_ids=list(range(8)))

    out = np.empty((B, T, D), np.float32)
    for core in range(8):
        b, hg = core // 2, core % 2
        raw = np.asarray(res.results[core]["out"], np.float32)
        raw = raw.reshape(P, 16, 2, 2, 130)                    # p,u,tt2,hp,130
        # t = u*256 + tt2*128 + p
        raw = raw.transpose(1, 2, 0, 3, 4).reshape(T, 2, 130)  # (t, hp, 130)
        # head h_local = 2*hp + j lives at cols [j*65, j*65+65)
        ov = np.empty((HPC, T, DV), np.float32)
        for hl in range(HPC):
            hp, j = hl // 2, hl % 2
            blk = raw[:, hp, j * 65:(j + 1) * 65]
            ov[hl] = blk[:, 0:DV] / blk[:, DV:DV + 1]
        out[b, hg * 2048:(hg + 1) * 2048, :] = ov.reshape(2048, 512)
    return out


# revision 13
# speedup vs baseline: 1.1067x; 1.0409x over previous
"""Multi-head linear attention (Performer/FAVOR+) Bass kernel for 8x TRN2 cores.

Sharding: 8 cores = 4 batches x 2 head-groups. Core c handles batch c//2 and
heads [4*(c%2), 4*(c%2)+4).

Math notes (exact rewrites of the reference, not approximations):
  - omega is sqrt(64) * orthogonal, so Omega @ Omega.T = 64*I. Hence
    0.5*||q||^2 = ||q @ Omega.T||^2 / 128: the squared-sum term is computed
    from xw itself and the plain q/k projections are never needed.
  - The per-row scale exp(-sq_t) on phi(q), the global 1/sqrt(128) scale and
    (approximately) the +EPS term all cancel in out = qkv[..,:64]/qkv[..,64],
    so the q-side feature map is just exp(+-xw).
  - The k-side scale rho_s = exp(-ksq_s) is folded into v1 = [v, 1]*rho so
    kp is also just exp(+-kxw).
  - The final divide (qkv[..,:64] / qkv[..,64]) runs on the HOST: the NEFF
    streams out raw [qkv_v | normalizer] per head in bf16.

Q-side projection computes only the 64 positive features per head (wqp2
packs two heads into the 128 stationary columns); exp(+x) and exp(-x) are
two full-partition ACT ops over the same PSUM tile, giving qpP=[h0+,h1+]
and qpM=[h0-,h1-] tiles. The qkv matmuls then use block-diagonal moving
operands kvBD_P/kvBD_M (built once per rep with cross-partition-offset ACT
copies) so one 128-contraction MM yields both heads' partial products.

DMA: descriptor generation on the issuing engine queue costs ~5ns per
descriptor, and a descriptor needs contiguous runs on BOTH ends. All DRAM
layouts are therefore window-blocked [p, c, w] (one contiguous run per
partition per DMA = 128 descriptors); SBUF tiles are flat per-partition
with computed column offsets. Loads split across the sync and gpsimd
queues. Output goes out in 4 batched p-major DMAs.

PSUM budget (8 banks): kxwv(2x2) + qx2(2) + kvacc(1); tail qkv units reuse
the kxwv/qx2 tags (2 banks each, 3 rotating, regions padded to bank size).
"""

import sys

import numpy as np

for _p in ("/opt/trn_rl_repo", "/root/.axon_site/_ro/trn_rl_repo"):
    try:
        import concourse  # noqa: F401
        break
    except ImportError:
        if _p not in sys.path:
            sys.path.insert(0, _p)

B, T, D, H = 4, 4096, 512, 8
DK = DV = 64
HPC = 4            # heads per core
NCH = 4            # f chunks (512 / 128)
P = 128
ST = T // P        # 32 s-tiles
TC = 8             # t chunks
TCW = T // TC      # 512

# window-blocked DRAM/SBUF layouts (shared host/kernel contract)
KWINS = [(0, 128), (128, 256), (256, 512), (512, 1536), (1536, 2560),
         (2560, 3584), (3584, 4096)]
QWINS = [(0, 512), (512, 1536), (1536, 2560), (2560, 3584), (3584, 4096)]


def _win_off(wins):
    offs = []
    base = 0
    for lo, hi in wins:
        offs.append(base)
        base += NCH * (hi - lo)
    return offs, base


KOFFS, KTOT = _win_off(KWINS)   # per-partition elems: 4*4096
QOFFS, QTOT = _win_off(QWINS)


def _col(wins, offs, c, col):
    """SBUF flat column offset for (chunk c, original column col)."""
    for (lo, hi), base in zip(wins, offs):
        if lo <= col < hi:
            return base + c * (hi - lo) + (col - lo)
    raise ValueError(col)


_CACHE = {}


def _build_program(reps=1):
    import concourse.mybir as mybir
    import concourse.tile as tile
    from concourse import bacc
    from contextlib import ExitStack

    dt = mybir.dt
    AF = mybir.ActivationFunctionType

    nc = bacc.Bacc("TRN2", target_bir_lowering=False, debug=False)

    qt_d = nc.dram_tensor("qt", [P * QTOT], dt.float16, kind="ExternalInput")
    kt_d = nc.dram_tensor("kt", [P * KTOT], dt.float16, kind="ExternalInput")
    vt_d = nc.dram_tensor("vt", [P * KTOT], dt.float16, kind="ExternalInput")
    wqp2_d = nc.dram_tensor("wqp2", [P * 2 * NCH * P], dt.float16,
                            kind="ExternalInput")
    wko_d = nc.dram_tensor("wko", [P * NCH * HPC * DK], dt.float16,
                           kind="ExternalInput")
    wv_d = nc.dram_tensor("wv", [P * NCH * HPC * DV], dt.float16,
                          kind="ExternalInput")
    # raw out, p-major: [p, unit(16), tt2(2), hp(2), 130] bf16
    out_d = nc.dram_tensor("out", [P * 16 * 520], dt.bfloat16,
                           kind="ExternalOutput")

    with tile.TileContext(nc) as tc, ExitStack() as ctx:
        const = ctx.enter_context(tc.tile_pool(name="const", bufs=1))
        work = ctx.enter_context(tc.tile_pool(name="work", bufs=3))
        psum = ctx.enter_context(tc.tile_pool(name="psum", bufs=1, space="PSUM"))
        for _rep in range(reps):
            _emit_body(nc, tc, const, work, psum, mybir, dt, AF,
                       qt_d, kt_d, vt_d, wqp2_d, wko_d, wv_d, out_d)

    nc.compile()
    return nc


def _emit_body(nc, tc, const, work, psum, mybir, dt, AF,
               qt_d, kt_d, vt_d, wqp2_d, wko_d, wv_d, out_d):
    if True:

        # persistent SBUF residents (flat window-blocked layouts)
        qt = const.tile([P, QTOT], dt.float16)
        kt = const.tile([P, KTOT], dt.float16)
        vt = const.tile([P, KTOT], dt.float16)
        wqp2 = const.tile([P, 2 * NCH * P], dt.float16)
        wko = const.tile([P, NCH * HPC * DK], dt.float16)
        wv = const.tile([P, NCH * HPC * DV], dt.float16)
        kvbd_p = const.tile([P, 2, 130], dt.bfloat16)
        kvbd_m = const.tile([P, 2, 130], dt.bfloat16)

        def flat_load(eng, dst, src_d, elem_off, elems):
            eng.dma_start(
                out=dst[:, elem_off:elem_off + elems],
                in_=src_d.ap()[P * elem_off:P * (elem_off + elems)].rearrange(
                    "(p x) -> p x", p=P),
            )

        # load order: weights and the first k/v windows unblock pair 0.
        # k/v windows take priority over qt (input DMA runs at the HBM
        # roofline through the first half of the loop; q-chunks only start
        # at pair 8, so qt windows are interleaved behind k/v).
        kw = [NCH * (hi - lo) for lo, hi in KWINS]
        qw = [NCH * (hi - lo) for lo, hi in QWINS]

        def kload(wi):
            flat_load(nc.sync, kt, kt_d, KOFFS[wi], kw[wi])

        def vload(wi):
            flat_load(nc.gpsimd, vt, vt_d, KOFFS[wi], kw[wi])

        def qload(wi, eng):
            flat_load(eng, qt, qt_d, QOFFS[wi], qw[wi])

        flat_load(nc.sync, wko, wko_d, 0, NCH * HPC * DK)
        flat_load(nc.gpsimd, wv, wv_d, 0, NCH * HPC * DV)
        kload(0); vload(0)
        kload(1); vload(1)
        kload(2); vload(2)
        flat_load(nc.sync, wqp2, wqp2_d, 0, 2 * NCH * P)
        kload(3); vload(3)
        qload(0, nc.gpsimd)
        kload(4); vload(4)
        qload(1, nc.sync)
        kload(5); vload(5)
        qload(2, nc.gpsimd)
        kload(6); vload(6)
        qload(3, nc.sync)
        qload(4, nc.gpsimd)

        nc.vector.memset(kvbd_p[:], 0.0)
        nc.vector.memset(kvbd_m[:], 0.0)

        # Single-bank PSUM accumulator: kv[h] at columns [h*65, h*65+65).
        # first_mm clears has_written at BANK granularity, so interleaved
        # per-head groups must NOT use start=True: memset the bank once and
        # accumulate from the first matmul.
        kv_big = psum.tile([P, HPC, DV + 1], dt.float32, tag="kvacc", bufs=1)
        nc.vector.memset(kv_big[:], 0.0)

        qp_tiles = []

        def emit_q_chunk(tcx):
            qx2 = psum.tile([P, 2, TCW], dt.float32, tag="qx2", bufs=1,
                            name="qx2")
            for hp in range(2):
                for c in range(NCH):
                    qoff = _col(QWINS, QOFFS, c, tcx * TCW)
                    nc.tensor.matmul(
                        qx2[:, hp, :],
                        wqp2[:, (hp * NCH + c) * P:(hp * NCH + c + 1) * P],
                        qt[:, qoff:qoff + TCW],
                        start=(c == 0), stop=(c == NCH - 1),
                    )
            qpp = work.tile([P, 2, TCW], dt.bfloat16, tag="qpp", bufs=TC,
                            name=f"qpp{tcx}")
            qpm = work.tile([P, 2, TCW], dt.bfloat16, tag="qpm", bufs=TC,
                            name=f"qpm{tcx}")
            nc.scalar.activation(qpp[:], qx2[:], AF.Exp, scale=1.0)
            nc.scalar.activation(qpm[:], qx2[:], AF.Exp, scale=-1.0)
            qp_tiles.append((qpp, qpm))

        # ---------------- phase KV (with q-projection work interleaved) -----
        # Engines execute their queues IN ORDER, so cross-engine dependencies
        # are software-pipelined: v1 (needs rho from ACT) is emitted one pair
        # late on DVE, the kv matmuls (need v1) one pair later still on PE.
        NP_ = ST // 2    # 16 pairs
        stage = {}       # pi -> dict of tiles

        def emit_v1(pi):
            st_ = stage[pi]
            v1 = work.tile([P, 2, HPC, DV + 1], dt.bfloat16, tag="v1",
                           name="v1")
            nc.vector.tensor_mul(
                v1[:, :, :, 0:DV], st_["v_ps"],
                st_["rho"][:].broadcast_to([P, 2, HPC, DV])
            )
            nc.vector.tensor_copy(v1[:, :, :, DV:DV + 1], st_["rho"][:])
            st_["v1"] = v1

        def emit_kv(pi):
            st_ = stage.pop(pi)
            for p_ in range(2):
                si = 2 * pi + p_
                for h in range(HPC):
                    nc.tensor.matmul(
                        kv_big[:, h, :], st_["kp"][:, p_, h, :],
                        st_["v1"][:, p_, h, :],
                        start=False, stop=(si == ST - 1),
                        skip_group_check=True,
                    )

        for pi in range(NP_):
            # kxw and v share PSUM banks: [..., 0:64] = kxw, 64:128 = v
            kxwv = psum.tile([P, 2, HPC, 2 * DK], dt.float32, tag="kxwv",
                             bufs=2, name="kxwv")
            kxw = kxwv[:, :, :, 0:DK]
            v_ps = kxwv[:, :, :, DK:2 * DK]
            for p_ in range(2):
                scol = (2 * pi + p_) * P
                for c in range(NCH):
                    koff = _col(KWINS, KOFFS, c, scol)
                    nc.tensor.matmul(
                        kxwv[:, p_, :, 0:DK], kt[:, koff:koff + P],
                        wko[:, c * 256:(c + 1) * 256],
                        start=(c == 0), stop=(c == NCH - 1),
                    )
                for c in range(NCH):
                    koff = _col(KWINS, KOFFS, c, scol)
                    nc.tensor.matmul(
                        kxwv[:, p_, :, DK:2 * DK], vt[:, koff:koff + P],
                        wv[:, c * 256:(c + 1) * 256],
                        start=(c == 0), stop=(c == NCH - 1),
                    )
            if pi >= 2:
                emit_kv(pi - 2)

            kp = work.tile([P, 2, HPC, 2 * DK], dt.bfloat16, tag="kp", bufs=3)
            nc.scalar.activation(kp[:, :, :, 0:DK], kxw, AF.Exp, scale=1.0)
            nc.scalar.activation(kp[:, :, :, DK:2 * DK], kxw, AF.Exp,
                                 scale=-1.0)

            kxw_sb = work.tile([P, 2, HPC, DK], dt.bfloat16, tag="kxwsb",
                               bufs=2)
            nc.vector.tensor_copy(kxw_sb[:], kxw)
            sqsc = work.tile([P, 2, HPC, DK], dt.bfloat16, tag="sqsc", bufs=2)
            nc.vector.tensor_mul(sqsc[:], kxw_sb[:], kxw_sb[:])
            ksqr = work.tile([P, 2, HPC, 1], dt.float32, tag="ksqr")
            nc.vector.reduce_sum(ksqr[:], sqsc[:], axis=mybir.AxisListType.X)
            rho = work.tile([P, 2, HPC, 1], dt.float32, tag="rho")
            nc.scalar.activation(rho[:], ksqr[:], AF.Exp, scale=-1.0 / 128.0)

            stage[pi] = {"v_ps": v_ps, "rho": rho, "kp": kp}
            if pi >= 1:
                emit_v1(pi - 1)

            if pi >= NP_ - TC:
                emit_q_chunk(pi - (NP_ - TC))

        emit_v1(NP_ - 1)
        emit_kv(NP_ - 2)
        emit_kv(NP_ - 1)

        # Block-diagonal kv for the 2-head qkv matmuls. kv_big rows: 0:64 =
        # plus feats, 64:128 = minus feats; columns per head. ACT copies
        # support cross-partition-offset placement.
        for hp in range(2):
            nc.scalar.copy(kvbd_p[0:64, hp, 0:65], kv_big[0:64, 2 * hp, :])
            nc.scalar.copy(kvbd_p[64:128, hp, 65:130],
                           kv_big[0:64, 2 * hp + 1, :])
            nc.scalar.copy(kvbd_m[0:64, hp, 0:65], kv_big[64:128, 2 * hp, :])
            nc.scalar.copy(kvbd_m[64:128, hp, 65:130],
                           kv_big[64:128, 2 * hp + 1, :])

        # ---------------- tail: qkv raw dump (normalize on host) ------------
        # Units of 2 t-tiles: psum [128, 2, 2, 256] = 2 banks, rotating
        # through the kxwv(x2) + qx2 tags; regions padded to 256 fp32 so
        # none crosses a PSUM bank boundary (start=True clears has_written
        # only for the bank containing the write start). Output stages into
        # 4-unit SBUF groups, DMA'd p-major (128 descriptors each).
        unit_tags = [("kxwv", 2), ("kxwv", 2), ("qx2", 1)]
        for tcx in range(TC):
            qpp, qpm = qp_tiles[tcx]
            for u in range(2):
                ui = tcx * 2 + u
                tg, bufs_ = unit_tags[ui % 3]
                qkv = psum.tile([P, 2, 2, 256], dt.float32, tag=tg,
                                bufs=bufs_, name="qkv")
                for tt2 in range(2):
                    ttsl = slice((u * 2 + tt2) * P, (u * 2 + tt2 + 1) * P)
                    for hp in range(2):
                        nc.tensor.matmul(
                            qkv[:, tt2, hp, 0:130], qpp[:, hp, ttsl],
                            kvbd_p[:, hp, :], start=True, stop=False,
                        )
                        nc.tensor.matmul(
                            qkv[:, tt2, hp, 0:130], qpm[:, hp, ttsl],
                            kvbd_m[:, hp, :], start=False, stop=True,
                        )
                o_sb = work.tile([P, 2, 2, 130], dt.bfloat16, tag="osb",
                                 bufs=6, name="osb")
                nc.vector.tensor_copy(o_sb[:], qkv[:, :, :, 0:130])
                out_eng = nc.gpsimd if ui % 2 == 0 else nc.sync
                out_eng.dma_start(
                    out=out_d.ap().rearrange(
                        "(p u x) -> p u x", p=P, u=16
                    )[:, ui:ui + 1, :],
                    in_=o_sb[:],
                )


def _get_program(reps=1):
    if reps not in _CACHE:
        _CACHE[reps] = _build_program(reps)
    return _CACHE[reps]


def _win_block(a2d, wins):
    """[512, T] f-major -> concat of per-window [p, c, w] blocks, flat."""
    a4 = a2d.reshape(NCH, P, a2d.shape[1])
    blocks = [
        np.ascontiguousarray(a4[:, :, lo:hi].transpose(1, 0, 2)).ravel()
        for lo, hi in wins
    ]
    return np.concatenate(blocks)


def _prep_core_inputs(query, value, key, wqo, wko, wv_w, core):
    b, hg = core // 2, core % 2
    hs = slice(hg * HPC, (hg + 1) * HPC)

    qT = query[b].T.astype(np.float16)                         # (512, 4096)
    kT = key[b].T.astype(np.float16)
    vT = value[b].T.astype(np.float16)

    wqo_c = wqo[hs]                                            # (4, 512, 64)
    # head-pair packing: [h_even 64 cols | h_odd 64 cols]
    wqp2 = np.stack([
        np.concatenate([wqo_c[2 * hp], wqo_c[2 * hp + 1]], axis=1)
        for hp in range(2)
    ])                                                         # (2, 512, 128)
    # -> [p, hp, c, m] flat
    wqp2 = np.ascontiguousarray(
        wqp2.reshape(2, NCH, P, P).transpose(2, 0, 1, 3)).ravel()
    wqp2 = wqp2.astype(np.float16)

    wko_c = np.concatenate(list(wko[hs]), axis=1)              # (512, 256)
    wko_c = np.ascontiguousarray(
        wko_c.reshape(NCH, P, HPC * DK).transpose(1, 0, 2)).ravel()
    wko_c = wko_c.astype(np.float16)
    wv_c = np.concatenate(list(wv_w[hs]), axis=1)              # (512, 256)
    wv_c = np.ascontiguousarray(
        wv_c.reshape(NCH, P, HPC * DV).transpose(1, 0, 2)).ravel()
    wv_c = wv_c.astype(np.float16)

    return {"qt": _win_block(qT, QWINS), "kt": _win_block(kT, KWINS),
            "vt": _win_block(vT, KWINS),
            "wqp2": wqp2, "wko": wko_c, "wv": wv_c}


def kernel(query, value, key, wq, wv, wk, omega):
    from concourse.bass_utils import run_bass_kernel_spmd

    query = np.asarray(query, np.float32)
    value = np.asarray(value, np.float32)
    key = np.asarray(key, np.float32)
    wq = np.asarray(wq, np.float32)
    wv = np.asarray(wv, np.float32)
    wk = np.asarray(wk, np.float32)
    omega = np.asarray(omega, np.float32)

    nc = _get_program()

    wqo = np.einsum("hfk,mk->hfm", wq, omega)                  # (8, 512, 64)
    wko = np.einsum("hfk,mk->hfm", wk, omega)

    in_maps = [
        _prep_core_inputs(query, value, key, wqo, wko, wv, core)
        for core in range(8)
    ]
    res = run_bass_kernel_spmd(nc, in_maps, core_ids=list(range(8)))

    out = np.empty((B, T, D), np.float32)
    for core in range(8):
        b, hg = core // 2, core % 2
        raw = np.asarray(res.results[core]["out"], np.float32)
        raw = raw.reshape(P, 16, 2, 2, 130)                    # p,u,tt2,hp,130
        # t = u*256 + tt2*128 + p
        raw = raw.transpose(1, 2, 0, 3, 4).reshape(T, 2, 130)  # (t, hp, 130)
        # head h_local = 2*hp + j lives at cols [j*65, j*65+65)
        ov = np.empty((HPC, T, DV), np.float32)
        for hl in range(HPC):
            hp, j = hl // 2, hl % 2
            blk = raw[:, hp, j * 65:(j + 1) * 65]
            ov[hl] = blk[:, 0:DV] / blk[:, DV:DV + 1]
        out[b, hg * 2048:(hg + 1) * 2048, :] = ov.reshape(2048, 512)
    return out
